# revision 14
# baseline (speedup 1.0000x reference)
"""VQ codebook argmin kernel for Trainium2 (8 NeuronCores, data-parallel).

Problem: latent [131072, 128] f32, coords [2048, 128] f32
         -> argmin_j ||latent_i - coords_j||^2  (int32 [131072])

Math: argmin_j (x2_i + c2_j - 2*cross_ij) = argmax_j (cross_ij - c2_j/2)
so per row we need the argmax of v = latent @ coords.T - h, h = |c|^2/2.

Device algorithm per 128-row tile (rows on partitions):
  1. PE: cross tile [128, 2048] in PSUM (4 matmuls, lhsT = latentT tile
     [128f x 128r], rhs = coordsT [128f x 2048c]).
  2. DVE: one fused custom-DVE pass: m = running-max-scan(cross - h)
     (PSUM+SBUF -> SBUF). The last scan element is the row max v*.
  3. ACT: one pass: out = Sign(v* - m), accum_out = sum = count of
     positions where the running max is still below v* = the index of the
     FIRST position achieving the max = argmax with jnp.argmin tie-break.
Host: shard latent rows 8 ways (pre-transposed per shard), replicate
coords; gather per-core counts and cast to int32.
"""

import numpy as np

import concourse.bass as bass
import concourse.bacc as bacc
import concourse.mybir as mybir
import concourse.tile as tile
import concourse.dve_ops as dve_ops
from concourse.dve_ops import DveOp
from concourse.dve_spec import Spec, Src0, Src1, AluOp, lower, _has_src1, scan
from concourse.dve_uop import DveOpSpec

P = 128          # partitions / rows per tile
D = 128          # feature dim
C = 2048         # n centroids
N_CORES = 8
FULL_ROWS = 131072
SHARD = FULL_ROWS // N_CORES      # 16384
MM_N = 512                        # fp32 moving-operand max

F32 = mybir.dt.float32

# ---------------------------------------------------------------- custom op
_OP_NAME = "SUB_SCANMAX_ANT"


def _register_scanmax_op() -> DveOp:
    """out[p, k] = max over j<=k of (in0[p, j] - in1[p, j]).

    Registered dynamically into dve_ops.OPS (shas computed at import, same
    process does both codegen and table-gen so the registry stays coherent).
    """
    for op in dve_ops.OPS:
        if op.name == _OP_NAME:
            return op
    spec = Spec(
        body=scan(AluOp.MAX, Src0 - Src1),
        reference=lambda in0, in1, s0, s1, imm2: np.maximum.accumulate(
            in0.astype(np.float32) - in1.astype(np.float32), axis=-1
        ),
    )
    row = 1 + len(dve_ops.OPS)
    shas = {
        ver: DveOpSpec(
            name=_OP_NAME, opcode=row, uops=lower(spec, ver=ver),
            rd1_en=_has_src1(spec),
        ).sha(ver)
        for ver in ("v3", "v4")
    }
    op = DveOp(_OP_NAME, spec, subdim=False, uops_sha=shas)
    dve_ops.OPS.append(op)
    dve_ops.CUSTOM_DVE_SPECS[_OP_NAME] = op.spec
    dve_ops._SUB_OPCODE_FOR_NAME[_OP_NAME] = row
    return op


SCANMAX = _register_scanmax_op()


# ---------------------------------------------------------------- kernel IR
def build_nc(n_tiles: int, variant: str = "fused", count_engine: str = "act"):
    """Build the per-core Bass program.

    n_tiles: number of 128-row tiles this core processes.
    variant: 'fused' (custom DVE op) or 'stock' (tensor_sub + tensor_tensor_scan).
    count_engine: 'act' (Sign+accum), 'dve' (STT is_lt + accum) fallback.
    """
    rows = n_tiles * P
    nc = bacc.Bacc("TRN2", target_bir_lowering=False, debug=False)
    latT = nc.dram_tensor("latT", [D, rows], F32, kind="ExternalInput").ap()
    coordsT = nc.dram_tensor("coordsT", [D, C], F32, kind="ExternalInput").ap()
    hb = nc.dram_tensor("hb", [P, C], F32, kind="ExternalInput").ap()
    if variant == "rank1":
        ones1 = nc.dram_tensor("ones1", [1, P], F32, kind="ExternalInput").ap()
        hneg1 = nc.dram_tensor("hneg1", [1, C], F32, kind="ExternalInput").ap()
    out = nc.dram_tensor("out", [P, n_tiles], F32, kind="ExternalOutput").ap()

    with tile.TileContext(nc) as tc:
        with (
            tc.tile_pool(name="const", bufs=1) as cpool,
            tc.tile_pool(name="scan", bufs=3) as mpool,
            tc.tile_pool(name="scr", bufs=2) as spool,
            tc.tile_pool(name="oacc", bufs=1) as opool,
            tc.tile_pool(name="ps", bufs=2, space="PSUM") as pspool,
        ):
            ct = cpool.tile([P, C], F32)
            nc.gpsimd.dma_start(out=ct[:], in_=coordsT)
            ht = cpool.tile([P, C], F32)
            nc.gpsimd.dma_start(out=ht[:], in_=hb)
            # whole latent shard stays SBUF-resident (64KB/partition)
            lat_all = cpool.tile([P, n_tiles * P], F32)
            nc.gpsimd.dma_start(out=lat_all[:], in_=latT)
            oacc = opool.tile([P, n_tiles], F32)

            if variant == "rank1":
                onest = cpool.tile([1, P], F32)
                nc.gpsimd.dma_start(out=onest[:], in_=ones1)
                hnt = cpool.tile([1, C], F32)
                nc.gpsimd.dma_start(out=hnt[:], in_=hneg1)

            # Self-loading fp32 matmuls (LDW struct) accept only ONE sync
            # wait. Re-write every matmul-read tile in place on ACT so all
            # matmul input deps collapse onto the single ACT proc tick.
            nc.scalar.copy(ct[:], ct[:])
            nc.scalar.copy(lat_all[:], lat_all[:])
            if variant == "rank1":
                nc.scalar.copy(onest[:], onest[:])
                nc.scalar.copy(hnt[:], hnt[:])

            for t in range(n_tiles):
                lat = lat_all[:, t * P:(t + 1) * P]
                ps = pspool.tile([P, C], F32)
                for k in range(C // MM_N):
                    sl = slice(k * MM_N, (k + 1) * MM_N)
                    nc.tensor.matmul(
                        ps[:, sl], lhsT=lat, rhs=ct[:, sl],
                        start=True, stop=(variant != "rank1"),
                    )
                    if variant == "rank1":
                        # accumulate ones^T @ (-h) so PSUM holds cross - h
                        nc.tensor.matmul(
                            ps[:, sl], lhsT=onest[:], rhs=hnt[:, sl],
                            start=False, stop=True,
                        )

                m = mpool.tile([P, C], F32)
                if variant == "fused":
                    nc.vector._custom_dve(SCANMAX, out=m[:], in0=ps[:], in1=ht[:])
                elif variant == "rank1":
                    nc.vector.tensor_tensor_scan(
                        out=m[:], data0=ps[:], data1=ht[:],
                        initial=-3.0e38,
                        op0=mybir.AluOpType.max, op1=mybir.AluOpType.bypass,
                    )
                else:
                    v = mpool.tile([P, C], F32, tag="vtmp")
                    nc.vector.tensor_sub(v[:], ps[:], ht[:])
                    nc.vector.tensor_tensor_scan(
                        out=m[:], data0=v[:], data1=v[:],
                        initial=-3.0e38,
                        op0=mybir.AluOpType.max, op1=mybir.AluOpType.bypass,
                    )

                vstar = m[:, C - 1:C]
                if count_engine == "act":
                    sgn = spool.tile([P, C], F32)
                    nc.scalar.activation(
                        out=sgn[:], in_=m[:],
                        func=mybir.ActivationFunctionType.Sign,
                        bias=vstar, scale=-1.0,
                        accum_out=oacc[:, t:t + 1],
                    )
                else:  # 'dve' fallback: exact ALU compare + accum on DVE
                    lt = spool.tile([P, C], F32)
                    nc.vector.scalar_tensor_tensor(
                        out=lt[:], in0=m[:], scalar=vstar, in1=m[:],
                        op0=mybir.AluOpType.is_lt,
                        op1=mybir.AluOpType.bypass,
                        accum_out=oacc[:, t:t + 1],
                    )

            nc.gpsimd.dma_start(out=out, in_=oacc[:])

    _strip_pe_self_waits(nc)
    # Bacc defers reg-alloc / wait-splitting to its compile pipeline, which
    # runs in finalize(); the bass2jax/axon exec path does not call it.
    nc.finalize()
    return nc


def _strip_pe_self_waits(nc):
    """Self-loading fp32 matmuls lower to an LDW struct that accepts only ONE
    sync wait. Tile emits a redundant same-engine (PE-sem) wait for PSUM-slot
    WAW reuse on top of the cross-engine reader-release wait; MM execution is
    strict-FIFO on PE (and LDW never touches PSUM/SBUF-writes), so the
    same-engine wait is timing-irrelevant. Drop PE-updated sems from matmul
    waits when more than one wait is present."""
    pe_sems = set()
    for blk in nc.m.functions[0].blocks:
        for i in blk.instructions:
            if getattr(i, "engine", None) == mybir.EngineType.PE and i.sync_info:
                for u in i.sync_info.on_update:
                    pe_sems.add(u.ant_name)
    for blk in nc.m.functions[0].blocks:
        for i in blk.instructions:
            if type(i).__name__ not in ("InstMatmult", "InstLdweights"):
                continue
            si = i.sync_info
            if not si or len(si.on_wait) <= 1:
                continue
            kept = [w for w in si.on_wait if w.ant_name not in pe_sems]
            if len(kept) != len(si.on_wait):
                assert kept, f"{i.name}: all waits were PE-self waits"
                si.on_wait = kept
                i.sync_info = si


# ---------------------------------------------------------------- host side
def _prep_core_inputs(latent: np.ndarray, coords: np.ndarray,
                      variant: str = "fused"):
    coords = np.asarray(coords, dtype=np.float32)
    latent = np.asarray(latent, dtype=np.float32)
    c2 = np.sum(coords * coords, axis=1, dtype=np.float32)
    h = (0.5 * c2).astype(np.float32)
    coordsT = np.ascontiguousarray(coords.T)                    # [128, 2048]
    hb = np.ascontiguousarray(np.broadcast_to(h[None, :], (P, C)))
    in_maps = []
    for i in range(N_CORES):
        shard = latent[i * SHARD:(i + 1) * SHARD]
        m = {
            "latT": np.ascontiguousarray(shard.T),              # [128, 16384]
            "coordsT": coordsT,
            "hb": hb,
        }
        if variant == "rank1":
            m["ones1"] = np.ones((1, P), np.float32)
            m["hneg1"] = np.ascontiguousarray(-h[None, :])
        in_maps.append(m)
    return in_maps


_NC_CACHE: dict = {}


def _get_nc(variant: str, count_engine: str):
    key = (variant, count_engine, SHARD // P)
    if key not in _NC_CACHE:
        _NC_CACHE[key] = build_nc(SHARD // P, variant, count_engine)
    return _NC_CACHE[key]


def run_on_cores(latent, coords, variant="fused", count_engine="act",
                 trace=False):
    from concourse.bass_utils import run_bass_kernel_spmd

    nc = _get_nc(variant, count_engine)
    in_maps = _prep_core_inputs(latent, coords, variant)
    res = run_bass_kernel_spmd(nc, in_maps, core_ids=list(range(N_CORES)),
                               trace=trace)
    shards = []
    for i in range(N_CORES):
        o = res.results[i]["out"]                    # [128, n_tiles] f32
        shards.append(np.rint(o).astype(np.int32).T.reshape(-1))
    return np.concatenate(shards), res


def kernel(latent: np.ndarray, coords: np.ndarray) -> np.ndarray:
    idx, _ = run_on_cores(latent, coords)
    return idx


# revision 18
# speedup vs baseline: 1.1846x; 1.1846x over previous
"""VQ codebook argmin kernel for Trainium2 (8 NeuronCores, data-parallel).

Problem: latent [131072, 128] f32, coords [2048, 128] f32
         -> argmin_j ||latent_i - coords_j||^2  (int32 [131072])

Math: argmin_j (x2_i + c2_j - 2*cross_ij) = argmax_j (cross_ij - c2_j/2)
so per row we need the argmax of v = latent @ coords.T - h, h = |c|^2/2.

Device algorithm per 128-row tile (rows on partitions):
  1. PE: cross tile [128, 2048] in PSUM (4 matmuls, lhsT = latentT tile
     [128f x 128r], rhs = coordsT [128f x 2048c]).
  2. DVE: one fused custom-DVE pass: m = running-max-scan(cross - h)
     (PSUM+SBUF -> SBUF). The last scan element is the row max v*.
  3. ACT: one pass: out = Sign(v* - m), accum_out = sum = count of
     positions where the running max is still below v* = the index of the
     FIRST position achieving the max = argmax with jnp.argmin tie-break.
Host: shard latent rows 8 ways (pre-transposed per shard), replicate
coords; gather per-core counts and cast to int32.
"""

import numpy as np

import concourse.bass as bass
import concourse.bacc as bacc
import concourse.mybir as mybir
import concourse.tile as tile
import concourse.dve_ops as dve_ops
from concourse.dve_ops import DveOp
from concourse.dve_spec import Spec, Src0, Src1, AluOp, lower, _has_src1, scan
from concourse.dve_uop import DveOpSpec

P = 128          # partitions / rows per tile
D = 128          # feature dim
C = 2048         # n centroids
N_CORES = 8
FULL_ROWS = 131072
SHARD = FULL_ROWS // N_CORES      # 16384
MM_N = 512                        # fp32 moving-operand max

F32 = mybir.dt.float32

# ---------------------------------------------------------------- custom op
_OP_NAME = "SUB_SCANMAX_ANT"


def _register_scanmax_op() -> DveOp:
    """out[p, k] = max over j<=k of (in0[p, j] - in1[p, j]).

    Registered dynamically into dve_ops.OPS (shas computed at import, same
    process does both codegen and table-gen so the registry stays coherent).
    """
    for op in dve_ops.OPS:
        if op.name == _OP_NAME:
            return op
    spec = Spec(
        body=scan(AluOp.MAX, Src0 - Src1),
        reference=lambda in0, in1, s0, s1, imm2: np.maximum.accumulate(
            in0.astype(np.float32) - in1.astype(np.float32), axis=-1
        ),
    )
    row = 1 + len(dve_ops.OPS)
    shas = {
        ver: DveOpSpec(
            name=_OP_NAME, opcode=row, uops=lower(spec, ver=ver),
            rd1_en=_has_src1(spec),
        ).sha(ver)
        for ver in ("v3", "v4")
    }
    op = DveOp(_OP_NAME, spec, subdim=False, uops_sha=shas)
    dve_ops.OPS.append(op)
    dve_ops.CUSTOM_DVE_SPECS[_OP_NAME] = op.spec
    dve_ops._SUB_OPCODE_FOR_NAME[_OP_NAME] = row
    return op


SCANMAX = _register_scanmax_op()


# ---------------------------------------------------------------- kernel IR
def build_nc(n_tiles: int, variant: str = "fused", count_engine: str = "act"):
    """Build the per-core Bass program.

    n_tiles: number of 128-row tiles this core processes.
    variant: 'fused' (custom DVE op) or 'stock' (tensor_sub + tensor_tensor_scan).
    count_engine: 'act' (Sign+accum), 'dve' (STT is_lt + accum) fallback.
    """
    rows = n_tiles * P
    nc = bacc.Bacc("TRN2", target_bir_lowering=False, debug=False)
    latT = nc.dram_tensor("latT", [D, rows], F32, kind="ExternalInput").ap()
    coordsT = nc.dram_tensor("coordsT", [D, C], F32, kind="ExternalInput").ap()
    hb = nc.dram_tensor("hb", [P, C], F32, kind="ExternalInput").ap()
    if variant == "rank1":
        ones1 = nc.dram_tensor("ones1", [1, P], F32, kind="ExternalInput").ap()
        hneg1 = nc.dram_tensor("hneg1", [1, C], F32, kind="ExternalInput").ap()
    out_dt = mybir.dt.uint32 if variant == "max8" else F32
    out = nc.dram_tensor("out", [P, n_tiles], out_dt, kind="ExternalOutput").ap()

    with tile.TileContext(nc) as tc:
        with (
            tc.tile_pool(name="const", bufs=1) as cpool,
            tc.tile_pool(name="scan", bufs=3) as mpool,
            tc.tile_pool(name="scr", bufs=2) as spool,
            tc.tile_pool(name="oacc", bufs=1) as opool,
            tc.tile_pool(name="ps", bufs=2, space="PSUM") as pspool,
        ):
            ct = cpool.tile([P, C], F32)
            nc.gpsimd.dma_start(out=ct[:], in_=coordsT)
            ht = cpool.tile([P, C], F32)
            nc.gpsimd.dma_start(out=ht[:], in_=hb)
            # whole latent shard stays SBUF-resident (64KB/partition)
            lat_all = cpool.tile([P, n_tiles * P], F32)
            nc.gpsimd.dma_start(out=lat_all[:], in_=latT)
            oacc = opool.tile([P, n_tiles], out_dt)

            if variant == "rank1":
                onest = cpool.tile([1, P], F32)
                nc.gpsimd.dma_start(out=onest[:], in_=ones1)
                hnt = cpool.tile([1, C], F32)
                nc.gpsimd.dma_start(out=hnt[:], in_=hneg1)

            # Self-loading fp32 matmuls (LDW struct) accept only ONE sync
            # wait. Re-write every matmul-read tile in place on ACT so all
            # matmul input deps collapse onto the single ACT proc tick.
            nc.scalar.copy(ct[:], ct[:])
            nc.scalar.copy(lat_all[:], lat_all[:])
            if variant == "rank1":
                nc.scalar.copy(onest[:], onest[:])
                nc.scalar.copy(hnt[:], hnt[:])

            for t in range(n_tiles):
                lat = lat_all[:, t * P:(t + 1) * P]
                ps = pspool.tile([P, C], F32)
                for k in range(C // MM_N):
                    sl = slice(k * MM_N, (k + 1) * MM_N)
                    nc.tensor.matmul(
                        ps[:, sl], lhsT=lat, rhs=ct[:, sl],
                        start=True, stop=(variant != "rank1"),
                    )
                    if variant == "rank1":
                        # accumulate ones^T @ (-h) so PSUM holds cross - h
                        nc.tensor.matmul(
                            ps[:, sl], lhsT=onest[:], rhs=hnt[:, sl],
                            start=False, stop=True,
                        )

                if variant == "max8":
                    v = mpool.tile([P, C], F32, tag="vtmp")
                    nc.vector.tensor_sub(v[:], ps[:], ht[:])
                    mx = spool.tile([P, 8], F32, tag="mx8")
                    nc.vector.max(out=mx[:], in_=v[:])
                    ix = spool.tile([P, 8], mybir.dt.uint32, tag="ix8")
                    nc.vector.max_index(ix[:], mx[:], v[:])
                    nc.vector.tensor_copy(oacc[:, t:t + 1], ix[:, 0:1])
                    continue

                m = mpool.tile([P, C], F32)
                if variant == "fused":
                    nc.vector._custom_dve(SCANMAX, out=m[:], in0=ps[:], in1=ht[:])
                elif variant == "rank1":
                    nc.vector.tensor_tensor_scan(
                        out=m[:], data0=ps[:], data1=ht[:],
                        initial=-3.0e38,
                        op0=mybir.AluOpType.max, op1=mybir.AluOpType.bypass,
                    )
                else:
                    v = mpool.tile([P, C], F32, tag="vtmp")
                    nc.vector.tensor_sub(v[:], ps[:], ht[:])
                    nc.vector.tensor_tensor_scan(
                        out=m[:], data0=v[:], data1=v[:],
                        initial=-3.0e38,
                        op0=mybir.AluOpType.max, op1=mybir.AluOpType.bypass,
                    )

                vstar = m[:, C - 1:C]
                if count_engine == "act":
                    sgn = spool.tile([P, C], F32)
                    nc.scalar.activation(
                        out=sgn[:], in_=m[:],
                        func=mybir.ActivationFunctionType.Sign,
                        bias=vstar, scale=-1.0,
                        accum_out=oacc[:, t:t + 1],
                    )
                else:  # 'dve' fallback: exact ALU compare + accum on DVE
                    lt = spool.tile([P, C], F32)
                    nc.vector.scalar_tensor_tensor(
                        out=lt[:], in0=m[:], scalar=vstar, in1=m[:],
                        op0=mybir.AluOpType.is_lt,
                        op1=mybir.AluOpType.bypass,
                        accum_out=oacc[:, t:t + 1],
                    )

            nc.gpsimd.dma_start(out=out, in_=oacc[:])

    _strip_pe_self_waits(nc)
    # Bacc defers reg-alloc / wait-splitting to its compile pipeline, which
    # runs in finalize(); the bass2jax/axon exec path does not call it.
    nc.finalize()
    return nc


def _strip_pe_self_waits(nc):
    """Self-loading fp32 matmuls lower to an LDW struct that accepts only ONE
    sync wait. Tile emits a redundant same-engine (PE-sem) wait for PSUM-slot
    WAW reuse on top of the cross-engine reader-release wait; MM execution is
    strict-FIFO on PE (and LDW never touches PSUM/SBUF-writes), so the
    same-engine wait is timing-irrelevant. Drop PE-updated sems from matmul
    waits when more than one wait is present."""
    pe_sems = set()
    for blk in nc.m.functions[0].blocks:
        for i in blk.instructions:
            if getattr(i, "engine", None) == mybir.EngineType.PE and i.sync_info:
                for u in i.sync_info.on_update:
                    pe_sems.add(u.ant_name)
    for blk in nc.m.functions[0].blocks:
        for i in blk.instructions:
            if type(i).__name__ not in ("InstMatmult", "InstLdweights"):
                continue
            si = i.sync_info
            if not si or len(si.on_wait) <= 1:
                continue
            kept = [w for w in si.on_wait if w.ant_name not in pe_sems]
            if len(kept) != len(si.on_wait):
                assert kept, f"{i.name}: all waits were PE-self waits"
                si.on_wait = kept
                i.sync_info = si


# ---------------------------------------------------------------- host side
def _prep_core_inputs(latent: np.ndarray, coords: np.ndarray,
                      variant: str = "fused"):
    coords = np.asarray(coords, dtype=np.float32)
    latent = np.asarray(latent, dtype=np.float32)
    c2 = np.sum(coords * coords, axis=1, dtype=np.float32)
    h = (0.5 * c2).astype(np.float32)
    coordsT = np.ascontiguousarray(coords.T)                    # [128, 2048]
    hb = np.ascontiguousarray(np.broadcast_to(h[None, :], (P, C)))
    in_maps = []
    for i in range(N_CORES):
        shard = latent[i * SHARD:(i + 1) * SHARD]
        m = {
            "latT": np.ascontiguousarray(shard.T),              # [128, 16384]
            "coordsT": coordsT,
            "hb": hb,
        }
        if variant == "rank1":
            m["ones1"] = np.ones((1, P), np.float32)
            m["hneg1"] = np.ascontiguousarray(-h[None, :])
        in_maps.append(m)
    return in_maps


_NC_CACHE: dict = {}


def _get_nc(variant: str, count_engine: str):
    key = (variant, count_engine, SHARD // P)
    if key not in _NC_CACHE:
        _NC_CACHE[key] = build_nc(SHARD // P, variant, count_engine)
    return _NC_CACHE[key]


def run_on_cores(latent, coords, variant="fused", count_engine="act",
                 trace=False):
    from concourse.bass_utils import run_bass_kernel_spmd

    nc = _get_nc(variant, count_engine)
    in_maps = _prep_core_inputs(latent, coords, variant)
    res = run_bass_kernel_spmd(nc, in_maps, core_ids=list(range(N_CORES)),
                               trace=trace)
    shards = []
    for i in range(N_CORES):
        o = res.results[i]["out"]                    # [128, n_tiles]
        o = np.rint(o) if o.dtype == np.float32 else o
        shards.append(o.astype(np.int32).T.reshape(-1))
    return np.concatenate(shards), res


def kernel(latent: np.ndarray, coords: np.ndarray) -> np.ndarray:
    idx, _ = run_on_cores(latent, coords)
    return idx


# revision 24
# speedup vs baseline: 1.2035x; 1.0160x over previous
"""VQ codebook argmin kernel for Trainium2 (8 NeuronCores, data-parallel).

Problem: latent [131072, 128] f32, coords [2048, 128] f32
         -> argmin_j ||latent_i - coords_j||^2  (int32 [131072])

Math: argmin_j (x2_i + c2_j - 2*cross_ij) = argmax_j (cross_ij - c2_j/2)
so per row we need the argmax of v = latent @ coords.T - h, h = |c|^2/2.

Device algorithm per 128-row tile (rows on partitions):
  1. PE: cross tile [128, 2048] in PSUM (4 matmuls, lhsT = latentT tile
     [128f x 128r], rhs = coordsT [128f x 2048c]).
  2. DVE: one fused custom-DVE pass: m = running-max-scan(cross - h)
     (PSUM+SBUF -> SBUF). The last scan element is the row max v*.
  3. ACT: one pass: out = Sign(v* - m), accum_out = sum = count of
     positions where the running max is still below v* = the index of the
     FIRST position achieving the max = argmax with jnp.argmin tie-break.
Host: shard latent rows 8 ways (pre-transposed per shard), replicate
coords; gather per-core counts and cast to int32.
"""

import numpy as np

import concourse.bass as bass
import concourse.bacc as bacc
import concourse.mybir as mybir
import concourse.tile as tile
import concourse.dve_ops as dve_ops
from concourse.dve_ops import DveOp
from concourse.dve_spec import Spec, Src0, Src1, AluOp, lower, _has_src1, scan
from concourse.dve_uop import DveOpSpec

P = 128          # partitions / rows per tile
D = 128          # feature dim
C = 2048         # n centroids
N_CORES = 8
FULL_ROWS = 131072
SHARD = FULL_ROWS // N_CORES      # 16384
MM_N = 512                        # fp32 moving-operand max

F32 = mybir.dt.float32

# ---------------------------------------------------------------- custom op
_OP_NAME = "SUB_SCANMAX_ANT"


def _register_scanmax_op() -> DveOp:
    """out[p, k] = max over j<=k of (in0[p, j] - in1[p, j]).

    Registered dynamically into dve_ops.OPS (shas computed at import, same
    process does both codegen and table-gen so the registry stays coherent).
    """
    for op in dve_ops.OPS:
        if op.name == _OP_NAME:
            return op
    spec = Spec(
        body=scan(AluOp.MAX, Src0 - Src1),
        reference=lambda in0, in1, s0, s1, imm2: np.maximum.accumulate(
            in0.astype(np.float32) - in1.astype(np.float32), axis=-1
        ),
    )
    row = 1 + len(dve_ops.OPS)
    shas = {
        ver: DveOpSpec(
            name=_OP_NAME, opcode=row, uops=lower(spec, ver=ver),
            rd1_en=_has_src1(spec),
        ).sha(ver)
        for ver in ("v3", "v4")
    }
    op = DveOp(_OP_NAME, spec, subdim=False, uops_sha=shas)
    dve_ops.OPS.append(op)
    dve_ops.CUSTOM_DVE_SPECS[_OP_NAME] = op.spec
    dve_ops._SUB_OPCODE_FOR_NAME[_OP_NAME] = row
    return op


SCANMAX = _register_scanmax_op()


# ---------------------------------------------------------------- kernel IR
def build_nc(n_tiles: int, variant: str = "fused", count_engine: str = "act"):
    """Build the per-core Bass program.

    n_tiles: number of 128-row tiles this core processes.
    variant: 'fused' (custom DVE op) or 'stock' (tensor_sub + tensor_tensor_scan).
    count_engine: 'act' (Sign+accum), 'dve' (STT is_lt + accum) fallback.
    """
    rows = n_tiles * P
    # float32r: same 4-byte storage as fp32, but the PE streams it at
    # 1 cycle/column instead of fp32's 4 (cost model: fp32 = "2 half-speed
    # matmuls"). Used for the matmul operands only; PSUM accumulation stays
    # fp32. 'f32r' in the variant name opts in.
    mm_dt = mybir.dt.float32r if variant.endswith("f32r") else F32
    nc = bacc.Bacc("TRN2", target_bir_lowering=False, debug=False)
    latT = nc.dram_tensor("latT", [D, rows], mm_dt, kind="ExternalInput").ap()
    coordsT = nc.dram_tensor("coordsT", [D, C], mm_dt, kind="ExternalInput").ap()
    hb = nc.dram_tensor("hb", [P, C], F32, kind="ExternalInput").ap()
    variant = variant.replace("_f32r", "")
    if variant == "rank1":
        ones1 = nc.dram_tensor("ones1", [1, P], F32, kind="ExternalInput").ap()
        hneg1 = nc.dram_tensor("hneg1", [1, C], F32, kind="ExternalInput").ap()
    out_dt = mybir.dt.uint32 if variant == "max8" else F32
    out = nc.dram_tensor("out", [P, n_tiles], out_dt, kind="ExternalOutput").ap()

    # Load the latent shard in independent chunks so compute starts after
    # chunk 0 instead of after the whole 8MB (single-queue SWDGE would
    # serialize ~300us of load ahead of the first matmul).
    n_chunks = max(1, min(16, n_tiles))
    while n_tiles % n_chunks:
        n_chunks -= 1
    tpc = n_tiles // n_chunks

    with tile.TileContext(nc) as tc:
        with (
            tc.tile_pool(name="const", bufs=1) as cpool,
            tc.tile_pool(name="lat", bufs=n_chunks) as lpool,
            tc.tile_pool(name="scan", bufs=3) as mpool,
            tc.tile_pool(name="scr", bufs=2) as spool,
            tc.tile_pool(name="oacc", bufs=1) as opool,
            tc.tile_pool(name="ps", bufs=2, space="PSUM") as pspool,
        ):
            ct = cpool.tile([P, C], mm_dt)
            nc.gpsimd.dma_start(out=ct[:], in_=coordsT)
            ht = cpool.tile([P, C], F32)
            nc.gpsimd.dma_start(out=ht[:], in_=hb)
            oacc = opool.tile([P, n_tiles], out_dt)

            if variant == "rank1":
                onest = cpool.tile([1, P], F32)
                nc.gpsimd.dma_start(out=onest[:], in_=ones1)
                hnt = cpool.tile([1, C], F32)
                nc.gpsimd.dma_start(out=hnt[:], in_=hneg1)

            # Self-loading fp32 matmuls (LDW struct) accept only ONE sync
            # wait. Re-write every matmul-read tile in place on ACT so all
            # matmul input deps collapse onto the single ACT proc tick.
            nc.scalar.copy(ct[:], ct[:])
            if variant == "rank1":
                nc.scalar.copy(onest[:], onest[:])
                nc.scalar.copy(hnt[:], hnt[:])

            # whole latent shard stays SBUF-resident (64KB/partition total),
            # one tile per chunk so Tile tracks chunk deps independently.
            lat_chunks = []
            for ci in range(n_chunks):
                lc = lpool.tile([P, tpc * P], mm_dt, tag="latc")
                nc.sync.dma_start(
                    out=lc[:], in_=latT[:, ci * tpc * P:(ci + 1) * tpc * P])
                nc.scalar.copy(lc[:], lc[:])
                lat_chunks.append(lc)

            for t in range(n_tiles):
                lat = lat_chunks[t // tpc][:, (t % tpc) * P:(t % tpc + 1) * P]
                ps = pspool.tile([P, C], F32)
                for k in range(C // MM_N):
                    sl = slice(k * MM_N, (k + 1) * MM_N)
                    nc.tensor.matmul(
                        ps[:, sl], lhsT=lat, rhs=ct[:, sl],
                        start=True, stop=(variant != "rank1"),
                    )
                    if variant == "rank1":
                        # accumulate ones^T @ (-h) so PSUM holds cross - h
                        nc.tensor.matmul(
                            ps[:, sl], lhsT=onest[:], rhs=hnt[:, sl],
                            start=False, stop=True,
                        )

                if variant == "max8":
                    v = mpool.tile([P, C], F32, tag="vtmp")
                    nc.vector.tensor_sub(v[:], ps[:], ht[:])
                    mx = spool.tile([P, 8], F32, tag="mx8")
                    nc.vector.max(out=mx[:], in_=v[:])
                    ix = spool.tile([P, 8], mybir.dt.uint32, tag="ix8")
                    nc.vector.max_index(ix[:], mx[:], v[:])
                    nc.vector.tensor_copy(oacc[:, t:t + 1], ix[:, 0:1])
                    continue

                m = mpool.tile([P, C], F32)
                if variant == "fused":
                    nc.vector._custom_dve(SCANMAX, out=m[:], in0=ps[:], in1=ht[:])
                elif variant == "rank1":
                    nc.vector.tensor_tensor_scan(
                        out=m[:], data0=ps[:], data1=ht[:],
                        initial=-3.0e38,
                        op0=mybir.AluOpType.max, op1=mybir.AluOpType.bypass,
                    )
                else:
                    v = mpool.tile([P, C], F32, tag="vtmp")
                    nc.vector.tensor_sub(v[:], ps[:], ht[:])
                    nc.vector.tensor_tensor_scan(
                        out=m[:], data0=v[:], data1=v[:],
                        initial=-3.0e38,
                        op0=mybir.AluOpType.max, op1=mybir.AluOpType.bypass,
                    )

                vstar = m[:, C - 1:C]
                if count_engine == "act":
                    sgn = spool.tile([P, C], F32)
                    nc.scalar.activation(
                        out=sgn[:], in_=m[:],
                        func=mybir.ActivationFunctionType.Sign,
                        bias=vstar, scale=-1.0,
                        accum_out=oacc[:, t:t + 1],
                    )
                else:  # 'dve' fallback: exact ALU compare + accum on DVE
                    lt = spool.tile([P, C], F32)
                    nc.vector.scalar_tensor_tensor(
                        out=lt[:], in0=m[:], scalar=vstar, in1=m[:],
                        op0=mybir.AluOpType.is_lt,
                        op1=mybir.AluOpType.bypass,
                        accum_out=oacc[:, t:t + 1],
                    )

            nc.gpsimd.dma_start(out=out, in_=oacc[:])

    _strip_pe_self_waits(nc)
    # Bacc defers reg-alloc / wait-splitting to its compile pipeline, which
    # runs in finalize(); the bass2jax/axon exec path does not call it.
    nc.finalize()
    return nc


def _strip_pe_self_waits(nc):
    """Self-loading fp32 matmuls lower to an LDW struct that accepts only ONE
    sync wait. Tile emits a redundant same-engine (PE-sem) wait for PSUM-slot
    WAW reuse on top of the cross-engine reader-release wait; MM execution is
    strict-FIFO on PE (and LDW never touches PSUM/SBUF-writes), so the
    same-engine wait is timing-irrelevant. Drop PE-updated sems from matmul
    waits when more than one wait is present."""
    pe_sems = set()
    for blk in nc.m.functions[0].blocks:
        for i in blk.instructions:
            if getattr(i, "engine", None) == mybir.EngineType.PE and i.sync_info:
                for u in i.sync_info.on_update:
                    pe_sems.add(u.ant_name)
    for blk in nc.m.functions[0].blocks:
        for i in blk.instructions:
            if type(i).__name__ not in ("InstMatmult", "InstLdweights"):
                continue
            si = i.sync_info
            if not si or len(si.on_wait) <= 1:
                continue
            kept = [w for w in si.on_wait if w.ant_name not in pe_sems]
            if len(kept) != len(si.on_wait):
                assert kept, f"{i.name}: all waits were PE-self waits"
                si.on_wait = kept
                i.sync_info = si


# ---------------------------------------------------------------- host side
def _prep_core_inputs(latent: np.ndarray, coords: np.ndarray,
                      variant: str = "fused"):
    coords = np.asarray(coords, dtype=np.float32)
    latent = np.asarray(latent, dtype=np.float32)
    c2 = np.sum(coords * coords, axis=1, dtype=np.float32)
    h = (0.5 * c2).astype(np.float32)
    coordsT = np.ascontiguousarray(coords.T)                    # [128, 2048]
    hb = np.ascontiguousarray(np.broadcast_to(h[None, :], (P, C)))
    in_maps = []
    for i in range(N_CORES):
        shard = latent[i * SHARD:(i + 1) * SHARD]
        m = {
            "latT": np.ascontiguousarray(shard.T),              # [128, 16384]
            "coordsT": coordsT,
            "hb": hb,
        }
        if variant == "rank1":
            m["ones1"] = np.ones((1, P), np.float32)
            m["hneg1"] = np.ascontiguousarray(-h[None, :])
        in_maps.append(m)
    return in_maps


_NC_CACHE: dict = {}


def _get_nc(variant: str, count_engine: str):
    key = (variant, count_engine, SHARD // P)
    if key not in _NC_CACHE:
        _NC_CACHE[key] = build_nc(SHARD // P, variant, count_engine)
    return _NC_CACHE[key]


def run_on_cores(latent, coords, variant="fused", count_engine="act",
                 trace=False):
    from concourse.bass_utils import run_bass_kernel_spmd

    nc = _get_nc(variant, count_engine)
    in_maps = _prep_core_inputs(latent, coords, variant)
    res = run_bass_kernel_spmd(nc, in_maps, core_ids=list(range(N_CORES)),
                               trace=trace)
    shards = []
    for i in range(N_CORES):
        o = res.results[i]["out"]                    # [128, n_tiles]
        o = np.rint(o) if o.dtype == np.float32 else o
        shards.append(o.astype(np.int32).T.reshape(-1))
    return np.concatenate(shards), res


def kernel(latent: np.ndarray, coords: np.ndarray) -> np.ndarray:
    idx, _ = run_on_cores(latent, coords)
    return idx
